# revision 1
# baseline (speedup 1.0000x reference)
"""GAT-style graph attention kernel for Trainium2 (Bass/Tile), 8-core SPMD.

Per graph b (one NeuronCore each, B=8):
    X  = H[b] @ W                      [N, U]
    s  = X @ a_1   (per-query logit)   [N, 1]
    n  = X @ a_2   (per-key logit)     [N, 1]
    E  = leaky_relu(s_i + n_j, 0.2)    [N, N]
    P  = exp(E) * A[b]                 (== exp(E + NEG*(1-A)), A in {0,1})
    out= relu((P @ X) / rowsum(P))     [N, U]

v2 strategy (vs the Prelu+Exp baseline):
  exp(leaky(s_i + n_j)) = max(exp(s_i + n_j), exp(0.2 s_i) * exp(0.2 n_j))
  - branch 1 (x1): ONE ACT pass per query tile: Exp(n_bcast + s_i bias)
  - branch 2 (x2): rank-1 product of PRECOMPUTED factors:
    z_b = exp(0.2 n) broadcast (f16), w_i = exp(0.2 s) per-partition scalar
    -> DVE tensor_scalar at 4x (1.27us vs 3.7us ACT pass)
  This halves ScalarE time per iteration; ACT's slack absorbs most of the
  PSUM->SBUF copies of the transposed P tiles, balancing both engines at
  ~6.5us/iter, under the DMA pace.
  DMA: A tiles stream mostly as SWDGE f32->f16 cast; a few tiles ride the
  HWDGE (sync) queue as raw f32 (two DGE frontends in parallel beat one:
  394 vs 320 GB/s in isolation). f32 tiles are masked directly by a 1x
  DVE tensor_tensor (no cast pass).
  Tail: x1/x2/max have no dependence on A, so the last iterations'
  transcendental work runs early; only mask+transpose+matmul trails the
  last A byte. Mask is emitted in halves so transposes start sooner.
"""

import numpy as np
from contextlib import ExitStack

import concourse.bass as bass
import concourse.bacc as bacc
import concourse.mybir as mybir
import concourse.tile as tile
from concourse.masks import make_identity

F32 = mybir.dt.float32
F16 = mybir.dt.float16

N_NODES = 4096
N_FEAT = 128
N_UNITS = 64
N_CORES = 8
LEAKY_SLOPE = 0.2

# iterations whose A tile arrives as raw f32 on the HWDGE (sync) queue
R32_ITERS = (5, 11, 19, 25)
# iterations computing p via the factorized (DVE-heavy) path; the rest use
# the Prelu+Exp (ACT-heavy) path. Mix balances both engines.
FACT_ITERS = (1, 3, 6, 8, 11, 14, 16, 19, 23, 26, 28, 31)


def build_nc(n_nodes=N_NODES):
    P = 128  # partitions
    U = N_UNITS
    F = N_FEAT
    n_t = n_nodes // P          # node tiles (32 full size)
    assert n_nodes % P == 0

    r32 = set(i for i in R32_ITERS if i < n_t)
    fact = set(i for i in FACT_ITERS if i < n_t)
    if n_t < 32:
        fact = set(i for i in range(n_t) if i % 3 == 1 and i not in r32)

    nc = bacc.Bacc(None)
    H_d = nc.declare_dram_parameter("H", [n_nodes, F], F32, isOutput=False)
    A_d = nc.declare_dram_parameter("A", [n_nodes, n_nodes], F32, isOutput=False)
    W_d = nc.declare_dram_parameter("W", [F, U], F32, isOutput=False)
    a1_d = nc.declare_dram_parameter("a_1", [U, 1], F32, isOutput=False)
    a2_d = nc.declare_dram_parameter("a_2", [U, 1], F32, isOutput=False)
    out_d = nc.declare_dram_parameter("out", [n_nodes, U], F32, isOutput=True)

    M = mybir.AluOpType
    AF = mybir.ActivationFunctionType

    with tile.TileContext(nc) as tc, ExitStack() as ctx:
        const = ctx.enter_context(tc.tile_pool(name="const", bufs=1))
        persist = ctx.enter_context(tc.tile_pool(name="persist", bufs=1))

        # Small weight loads first (they gate the first prep matmuls),
        # then H chunks; A prefetch follows in the stream.
        W_sb = const.tile([F, U], F16)
        nc.gpsimd.dma_start(W_sb[:], W_d[:])
        a1_sb = const.tile([U, 1], F16)
        nc.gpsimd.dma_start(a1_sb[:], a1_d[:])
        a2_sb = const.tile([U, 1], F32)
        nc.sync.dma_start(a2_sb[:], a2_d[:])

        HCH = max(1, n_t // 4)

        ident16 = const.tile([P, P], F16)
        make_identity(nc, ident16[:])

        # a2 broadcast along free dim: a2b[u, c] = a2[u]
        a2b = const.tile([U, P], F16)
        nc.vector.memset(a2b[:], 1.0)
        nc.vector.tensor_scalar_mul(a2b[:], a2b[:], a2_sb[:, 0:1])

        # persistent per-graph tensors
        n_bcast = persist.tile([P, n_nodes], F32)     # n[j] bcast over partitions
        z_b = persist.tile([P, n_nodes], F16)         # exp(0.2 n[j]) bcast
        XT_sb = persist.tile([U, n_nodes], F16)       # X^T (u on partitions)
        Xp_sb = persist.tile([P, n_t * (U + 1)], F16)  # X' tiles [X_t | 1]
        s_sb = persist.tile([P, n_t], F32)            # s column per query tile
        w_sb = persist.tile([P, n_t], F32)            # exp(0.2 s)
        dinv_sb = persist.tile([P, n_t], F32)
        nc.vector.memset(Xp_sb[:], 1.0)

        # A prefetch pools opened up-front so the first loads are issued
        # ahead of prep in queue order (they only depend on DRAM and overlap
        # the whole prep phase on the DMA engines).
        apool = ctx.enter_context(tc.tile_pool(name="apool", bufs=4))
        apool32 = ctx.enter_context(tc.tile_pool(name="apool32", bufs=1))
        N_EARLY_A = min(5, n_t)
        early_a = {}

        def load_a(it):
            if it in r32:
                a_t = apool32.tile([P, n_nodes], F32, tag="a32")
                nc.sync.dma_start(a_t[:], A_d[it * P:(it + 1) * P, :])
            else:
                a_t = apool.tile([P, n_nodes], F16, tag="a16")
                nc.gpsimd.dma_start(a_t[:], A_d[it * P:(it + 1) * P, :])
            return a_t

        # ---------------- prep: X, X^T, s, z_b, n_bcast ----------------
        with tc.tile_pool(name="hpool", bufs=1) as hpool, \
             tc.tile_pool(name="prep", bufs=6) as prep, \
             tc.tile_pool(name="prep_ps", bufs=2, space="PSUM") as prep_ps, \
             tc.tile_pool(name="prep_ps1", bufs=2, space="PSUM") as prep_ps1:

            h_chunks = {}
            for c in range(0, n_t, HCH):
                hc = hpool.tile([P, HCH * F], F16, tag=f"h_all{c}")
                nc.gpsimd.dma_start(
                    hc[:].rearrange("p (t f) -> p t f", f=F),
                    H_d[c * P:(c + HCH) * P, :].rearrange(
                        "(t p) f -> p t f", p=P))
                h_chunks[c] = hc

            # A prefetch starts once H is queued (overlaps prep compute)
            for it in range(N_EARLY_A):
                early_a[it] = load_a(it)

            QB = 4 if n_t % 4 == 0 else 2
            s_tiles = {}
            for t2 in range(0, n_t, QB):
                hT_ps = prep_ps.tile([P, QB * P], F16, tag="hT_ps")
                for k in range(QB):
                    t = t2 + k
                    hc = h_chunks[(t // HCH) * HCH]
                    nc.tensor.transpose(hT_ps[:, k * P:k * P + F],
                                        hc[:, (t % HCH) * F:(t % HCH + 1) * F],
                                        ident16[:])
                hT_sb = prep.tile([F, QB * P], F16)
                nc.scalar.copy(hT_sb[:], hT_ps[:F, 0:QB * P])
                # X^T tiles: [U, node QB*128]
                xT_ps = prep_ps.tile([U, QB * P], F32, tag="xps")
                nc.tensor.matmul(xT_ps[:], W_sb[:], hT_sb[:], start=True, stop=True)
                if (t2 // QB) % 2 == 0:
                    nc.scalar.copy(XT_sb[:, t2 * P:(t2 + QB) * P], xT_ps[:])
                else:
                    nc.vector.tensor_copy(XT_sb[:, t2 * P:(t2 + QB) * P], xT_ps[:])
                # s[p, t] = (X @ a1)[t*128+p]
                s_q = prep_ps1.tile([P, QB], F32, tag="s_q")
                for k in range(QB):
                    nc.tensor.matmul(s_q[:, k:k + 1],
                                     XT_sb[:, (t2 + k) * P:(t2 + k + 1) * P],
                                     a1_sb[:], start=True, stop=True)
                s_sb_q = persist.tile([P, QB], F32, tag=f"s{t2}")
                nc.vector.tensor_copy(s_sb_q[:], s_q[:])
                s_tiles[t2] = s_sb_q
                nc.vector.tensor_copy(s_sb[:, t2:t2 + QB], s_q[:])
                # n_bcast[p, slice] = n[slice] broadcast over partitions
                nb_ps = prep_ps.tile([P, QB * P], F32, tag="nb_ps")
                nc.tensor.matmul(nb_ps[:], a2b[:],
                                 XT_sb[:, t2 * P:(t2 + QB) * P],
                                 start=True, stop=True)
                nc.vector.tensor_copy(n_bcast[:, t2 * P:(t2 + QB) * P],
                                      nb_ps[:])
                # z_b = exp(0.2 n) straight from PSUM on ACT
                nc.scalar.activation(z_b[:, t2 * P:(t2 + QB) * P], nb_ps[:],
                                     AF.Exp, scale=LEAKY_SLOPE)

            # X tiles for the H_cap matmuls, rebuilt from X^T off the
            # critical path (overlaps the start of the main loop).
            for t in range(n_t):
                x_ps = prep_ps.tile([P, U], F16, tag="xps")
                nc.tensor.transpose(x_ps[:, 0:U],
                                    XT_sb[:, t * P:(t + 1) * P],
                                    ident16[0:U, 0:U])
                nc.vector.tensor_copy(Xp_sb[:, t * (U + 1):t * (U + 1) + U],
                                      x_ps[:])
            # w = exp(0.2 s) per-partition scalars
            nc.scalar.activation(w_sb[:], s_sb[:], AF.Exp, scale=LEAKY_SLOPE)

        # ---------------- main loop over query tiles ----------------
        GROUP = 16                     # transposes per PSUM tile (2 banks)
        n_groups = (n_t + GROUP - 1) // GROUP
        LOOK = 2                       # x1/x2/p production lookahead (iters)

        with tc.tile_pool(name="x1pool", bufs=2) as x1pool, \
             tc.tile_pool(name="x2pool", bufs=2) as x2pool, \
             tc.tile_pool(name="ppool", bufs=LOOK + 2) as ppool, \
             tc.tile_pool(name="pmpool", bufs=2) as pmpool, \
             tc.tile_pool(name="ptpool", bufs=4) as ptpool, \
             tc.tile_pool(name="outpool", bufs=3) as outpool, \
             tc.tile_pool(name="psT", bufs=3, space="PSUM") as psT, \
             tc.tile_pool(name="psAcc", bufs=2, space="PSUM") as psAcc:

            p_tiles = {}
            acc_tiles = {}

            def produce(it):
                s_bias = s_tiles[(it // QB) * QB][:, it % QB:it % QB + 1]
                p_t = ppool.tile([P, n_nodes], F16, tag="p")
                # first iterations: chunk the first ACT pass per prep quad so
                # it starts as soon as each n_bcast quad lands (head latency)
                chunks = [(0, n_nodes)]
                if it in fact:
                    # factorized: exp(leaky(s+n)) = max(exp(s+n),
                    #   exp(0.2 s) * exp(0.2 n)) -- one ACT pass + DVE ts/max
                    x1 = x1pool.tile([P, n_nodes], F16, tag="x1")
                    for lo, hi in chunks:
                        nc.scalar.activation(x1[:, lo:hi], n_bcast[:, lo:hi],
                                             AF.Exp, bias=s_bias)
                    x2 = x2pool.tile([P, n_nodes], F16, tag="x2")
                    nc.vector.tensor_scalar_mul(x2[:], z_b[:],
                                                w_sb[:, it:it + 1])
                    nc.vector.tensor_max(p_t[:], x1[:], x2[:])
                else:
                    # ACT-heavy: Prelu then Exp (both on ScalarE, no DVE)
                    el = x1pool.tile([P, n_nodes], F16, tag="x1")
                    for lo, hi in chunks:
                        nc.scalar.activation(el[:, lo:hi], n_bcast[:, lo:hi],
                                             AF.Prelu, bias=s_bias, scale=1.0,
                                             alpha=LEAKY_SLOPE)
                    nc.scalar.activation(p_t[:], el[:], AF.Exp)
                p_tiles[it] = p_t

            def consume(it):
                if it in early_a:
                    a_t = early_a.pop(it)
                else:
                    a_t = load_a(it)
                p_t = p_tiles.pop(it)
                fine = it >= n_t - 2   # tail iterations: 8-block pipelining
                pm_t = pmpool.tile([P, n_nodes], F16, tag="pm")
                half = n_nodes // 2
                if not fine:
                    # masked P, in halves so group transposes start earlier
                    for hf in range(2):
                        nc.vector.tensor_mul(
                            pm_t[:, hf * half:(hf + 1) * half],
                            p_t[:, hf * half:(hf + 1) * half],
                            a_t[:, hf * half:(hf + 1) * half])

                # transpose P_m 128x128 blocks -> PSUM, copy groups to SBUF
                acc_ps = psAcc.tile([P, U + 1], F32, tag="acc_ps")
                for g in range(n_groups):
                    k_n = min(GROUP, n_t - g * GROUP)
                    pt_ps = psT.tile([P, GROUP * P], F16, tag="pt_ps")
                    for half_g in range(2 if fine else 1):
                        if fine:
                            lo = g * GROUP * P + half_g * (GROUP // 2) * P
                            hi = lo + (GROUP // 2) * P
                            nc.vector.tensor_mul(pm_t[:, lo:hi], p_t[:, lo:hi],
                                                 a_t[:, lo:hi])
                            ks = range(half_g * (GROUP // 2),
                                       min(k_n, (half_g + 1) * (GROUP // 2)))
                        else:
                            ks = range(k_n)
                        for k in ks:
                            jt = g * GROUP + k
                            nc.tensor.transpose(pt_ps[:, k * P:(k + 1) * P],
                                                pm_t[:, jt * P:(jt + 1) * P],
                                                ident16[:])
                    pt_sb = ptpool.tile([P, GROUP * P], F16, tag="pt_sb")
                    w_n = k_n * P
                    if fine:
                        # split the copy across both engines in the tail
                        nc.scalar.copy(pt_sb[:, 0:w_n // 2], pt_ps[:, 0:w_n // 2])
                        nc.vector.tensor_copy(pt_sb[:, w_n // 2:w_n],
                                              pt_ps[:, w_n // 2:w_n])
                    else:
                        nc.vector.tensor_copy(pt_sb[:, 0:w_n], pt_ps[:, 0:w_n])
                    # H_cap accumulation for this group's j tiles
                    for k in range(k_n):
                        jt = g * GROUP + k
                        nc.tensor.matmul(
                            acc_ps[:], pt_sb[:, k * P:(k + 1) * P],
                            Xp_sb[:, jt * (U + 1):(jt + 1) * (U + 1)],
                            start=(jt == 0), stop=(jt == n_t - 1))

                # reciprocal now: the chain just completed, and putting it
                # here keeps it ahead of the next iteration's DVE queue so
                # the deferred ACT relu never waits on it.
                nc.vector.reciprocal(dinv_sb[:, it:it + 1], acc_ps[:, U:U + 1])
                acc_tiles[it] = acc_ps

            def emit_out(it):
                # out = relu(H_cap[:, :U] / H_cap[:, U]) -- relu+scale on ACT.
                # Runs one iteration late so no engine stalls on the chain.
                acc_ps = acc_tiles.pop(it)
                out_t = outpool.tile([P, U], F32)
                nc.scalar.activation(out_t[:], acc_ps[:, 0:U], AF.Relu,
                                     scale=dinv_sb[:, it:it + 1])
                nc.sync.dma_start(out_d[it * P:(it + 1) * P, :], out_t[:])

            for it in range(n_t + LOOK + 1):
                if it < n_t:
                    produce(it)
                if LOOK <= it < n_t + LOOK:
                    ct = it - LOOK
                    consume(ct)
                    if ct >= n_t - 2:
                        emit_out(ct)
                if LOOK < it < n_t + LOOK - 1:
                    emit_out(it - LOOK - 1)

    nc.compile()
    return nc


_NC_CACHE = {}


def _get_nc(n_nodes=N_NODES):
    if n_nodes not in _NC_CACHE:
        _NC_CACHE[n_nodes] = build_nc(n_nodes)
    return _NC_CACHE[n_nodes]


def kernel(H, A, W, a_1, a_2):
    """Full inputs in, full output out. Shards batch across 8 NeuronCores."""
    import os
    # The axon trace path needs antenv.axon_hooks, which this image lacks;
    # make sure an inherited BASS_TRACE can't route us there.
    os.environ["BASS_NEVER_TRACE"] = "1"
    from concourse.bass_utils import run_bass_kernel_spmd

    B = H.shape[0]
    assert B == N_CORES
    nc = _get_nc(H.shape[1])
    in_maps = [
        {
            "H": np.ascontiguousarray(H[b], dtype=np.float32),
            "A": np.ascontiguousarray(A[b], dtype=np.float32),
            "W": np.ascontiguousarray(W, dtype=np.float32),
            "a_1": np.ascontiguousarray(a_1, dtype=np.float32),
            "a_2": np.ascontiguousarray(a_2, dtype=np.float32),
        }
        for b in range(B)
    ]
    res = run_bass_kernel_spmd(nc, in_maps, core_ids=list(range(N_CORES)))
    out = np.stack([res.results[b]["out"] for b in range(B)]).astype(np.float32)
    return out

